# revision 21
# baseline (speedup 1.0000x reference)
"""Lower-triangular matvec y = tril(W) @ x on 8 TRN2 NeuronCores.

Strategy (memory-bound problem; minimize + balance HBM traffic):
  - Split the 8192 rows into 64 row-tiles of 128. Row-tile t only needs
    columns [0, 128*(t+1)) of W (lower triangle).
  - Core k takes row-tiles {8g + k : g = 0..7} — one tile from each
    "width group" g, padded to the group max width 1024*(g+1). Every
    core therefore reads an identical 18.9MB (vs 32MB for naive row
    blocks), perfectly balanced, with SPMD-identical shapes.
  - The host applies the tril mask (zeros in the pad region) and stores
    each 128x128 tile TRANSPOSED in a [128, 36864] buffer so that
    (a) any chunk of tiles is a perfectly strided 2D DMA with 4KB
        contiguous runs per partition, and
    (b) each tile is directly usable as matmul lhsT (contraction j on
        partitions), letting the tensor engine accumulate y in PSUM.
  - Kernel: 36 DMA chunks of 512KB double-buffered, 288 matmuls
    [128jx128i]^T @ [128j x 1] accumulating 8 PSUM [128,1] partials,
    8 PSUM->SBUF copies, one [128,8] output DMA. No collectives needed;
    the host reassembles y.
"""

import os

import numpy as np

P = 128
N = 8192
NCORES = 8
NSEG = 8  # row-tiles (segments) per core
NTILEC = 64  # column tiles over full width
TOTAL_TILES = sum(8 * (g + 1) for g in range(NSEG))  # 288
WT_COLS = TOTAL_TILES * P  # 36864
DMA_CHUNK = 8  # 128x128 tiles per DMA (512KB)

_compiled = None
last_results = None  # BassKernelResults of the most recent run (for test.py)


def _build_program():
    import concourse.bacc as bacc
    import concourse.mybir as mybir
    from concourse.tile import TileContext

    f32 = mybir.dt.float32
    # Bacc (not raw Bass): its compile() runs move_matmul_waits_to_ldweights
    # and generate_event_semaphores, which TRN2's 1-wait-per-instruction
    # ISA constraint requires for Tile-scheduled programs.
    nc = bacc.Bacc(None, target_bir_lowering=False)
    wt = nc.declare_dram_parameter("wt", [P, WT_COLS], f32, isOutput=False)
    xr = nc.declare_dram_parameter("xr", [P, NTILEC], f32, isOutput=False)
    y = nc.declare_dram_parameter("y", [P, NSEG], f32, isOutput=True)

    with TileContext(nc) as tc:
        with (
            tc.tile_pool(name="xpool", bufs=1) as xpool,
            tc.tile_pool(name="wpool", bufs=36) as wpool,
            tc.tile_pool(name="ypool", bufs=1) as ypool,
            tc.tile_pool(name="psum", bufs=8, space="PSUM") as psum_pool,
        ):
            x_sb = xpool.tile([P, NTILEC], f32)
            nc.sync.dma_start(out=x_sb, in_=xr[:, :])
            y_sb = ypool.tile([P, NSEG], f32)

            ps_tiles = [
                psum_pool.tile([P, 1], f32, tag="ps", name=f"ps{g}")
                for g in range(NSEG)
            ]

            t = 0  # global 128x128 tile counter into wt
            for g in range(NSEG):
                C = 8 * (g + 1)  # column tiles in this segment
                ps = ps_tiles[g]
                for cc in range(0, C, DMA_CHUNK):
                    w_sb = wpool.tile([P, DMA_CHUNK * P], f32)
                    nc.sync.dma_start(
                        out=w_sb, in_=wt[:, P * t : P * (t + DMA_CHUNK)]
                    )
                    for u in range(DMA_CHUNK):
                        c = cc + u
                        nc.tensor.matmul(
                            ps,
                            w_sb[:, P * u : P * (u + 1)],
                            x_sb[:, c : c + 1],
                            start=(c == 0),
                            stop=(c == C - 1),
                        )
                    t += DMA_CHUNK
                nc.scalar.copy(y_sb[:, g : g + 1], ps)

            nc.sync.dma_start(out=y[:, :], in_=y_sb)

    nc.finalize()  # Bacc.compile(): wait splitting, reg alloc, etc.
    return nc


def _pack_core(W, k):
    """Build wt [128, 36864] for core k: tiles (8g+k) transposed, tril-masked."""
    segs = []
    for g in range(NSEG):
        i0 = P * (8 * g + k)
        width = 1024 * (g + 1)
        block = W[i0 : i0 + P, :width]
        cols = np.arange(width)[None, :]
        rows = (i0 + np.arange(P))[:, None]
        b = np.where(cols <= rows, block, np.float32(0.0)).astype(np.float32)
        c = width // P
        segs.append(b.reshape(P, c, P).transpose(2, 1, 0).reshape(P, c * P))
    return np.ascontiguousarray(np.concatenate(segs, axis=1))


def kernel(x, W, _trace=False, _trace_kwargs=None):
    global _compiled, last_results
    from concourse.bass_utils import run_bass_kernel_spmd

    x = np.asarray(x, dtype=np.float32)
    W = np.asarray(W, dtype=np.float32)

    if _compiled is None:
        _compiled = _build_program()

    xr = np.ascontiguousarray(x.reshape(NTILEC, P).T)
    in_maps = [{"wt": _pack_core(W, k), "xr": xr} for k in range(NCORES)]

    kw = {}
    if _trace:
        kw = dict(trace=True, **(_trace_kwargs or {}))
    res = run_bass_kernel_spmd(_compiled, in_maps, list(range(NCORES)), **kw)
    last_results = res

    y = np.empty(N, dtype=np.float32)
    for k in range(NCORES):
        yk = res.results[k]["y"]  # [128, 8]
        for g in range(NSEG):
            i0 = P * (8 * g + k)
            y[i0 : i0 + P] = yk[:, g]
    return y


# revision 23
# speedup vs baseline: 1.7251x; 1.7251x over previous
"""Lower-triangular matvec y = tril(W) @ x on 8 TRN2 NeuronCores.

Strategy (memory-bound problem; minimize + balance HBM traffic):
  - Split the 8192 rows into 64 row-tiles of 128. Row-tile t only needs
    columns [0, 128*(t+1)) of W (lower triangle).
  - Core k takes row-tiles {8g + k : g = 0..7} — one from each "width
    group" g, padded to the group max width 1024*(g+1). Every core
    therefore reads an identical 18.9MB (vs 32MB for naive row blocks),
    perfectly balanced, with SPMD-identical shapes.
  - The host applies the tril mask (zeros in the pad region) and stores
    tiles TRANSPOSED (contraction j on partitions), packed column-major:
    for each column-tile c, the tiles of all active segments g >= c//8
    sit contiguously. One DMA per c, perfectly strided.
  - PE usage: x columns are the STATIONARY operand (128-element weight
    loads) and W tiles stream as the moving operand, so each matmul does
    up to 512 columns of work per (tiny) weight load:
        ps[0, 128g + i] += sum_j x[128c+j] * W[128(8g+k)+i, 128c+j]
    96 matmuls total, accumulating y into a [1, 1024] PSUM row (two
    banks, matmuls never cross the 512 boundary).
  - 2 PSUM->SBUF copies, one [1, 1024] output DMA; host reassembles y.
"""

import numpy as np

P = 128
N = 8192
NCORES = 8
NSEG = 8  # row-tiles (segments) per core
NTILEC = 64  # column tiles over full width
# column-major packing: chunk c holds tiles for segs m..7, m = c//8
CHUNK_TILES = [NSEG - (c // NSEG) for c in range(NTILEC)]
CHUNK_OFF = np.concatenate([[0], np.cumsum(CHUNK_TILES)]).astype(int)  # in tiles
WT_COLS = int(CHUNK_OFF[-1]) * P  # 288 tiles * 128 = 36864

_compiled = None
last_results = None  # BassKernelResults of the most recent run (for test.py)


def _build_program():
    import concourse.bacc as bacc
    import concourse.mybir as mybir
    from concourse.tile import TileContext

    f32 = mybir.dt.float32
    # Bacc (not raw Bass): its compile() runs move_matmul_waits_to_ldweights
    # and generate_event_semaphores, which TRN2's 1-wait-per-instruction
    # ISA constraint requires for Tile-scheduled programs.
    nc = bacc.Bacc(None, target_bir_lowering=False)
    wt = nc.declare_dram_parameter("wt", [P, WT_COLS], f32, isOutput=False)
    xr = nc.declare_dram_parameter("xr", [P, NTILEC], f32, isOutput=False)
    y = nc.declare_dram_parameter("y", [1, NSEG * P], f32, isOutput=True)

    with TileContext(nc) as tc:
        with (
            tc.tile_pool(name="xpool", bufs=1) as xpool,
            tc.tile_pool(name="wpool", bufs=8) as wpool,
            tc.tile_pool(name="ypool", bufs=1) as ypool,
            tc.tile_pool(name="psum", bufs=1, space="PSUM") as psum_pool,
        ):
            x_sb = xpool.tile([P, NTILEC], f32)
            nc.sync.dma_start(out=x_sb, in_=xr[:, :])
            y_sb = ypool.tile([1, NSEG * P], f32)

            ps = psum_pool.tile([1, NSEG * P], f32)  # two PSUM banks

            for c in range(NTILEC):
                m = c // NSEG  # first active segment
                w = CHUNK_TILES[c] * P  # chunk width (tiles for segs m..7)
                # all 64 chunks get their own buffer (no reuse waits);
                # same-m chunks share a tag so slots are sized exactly.
                w_sb = wpool.tile([P, w], f32, tag=f"w{m}", bufs=NSEG, name=f"w_{c}")
                nc.sync.dma_start(
                    out=w_sb, in_=wt[:, P * CHUNK_OFF[c] : P * CHUNK_OFF[c + 1]]
                )
                # matmuls: out[0, 128g + i] over active segs; split at the
                # PSUM bank boundary (f32 bank = 512 elems).
                for lo, hi in ((m * P, 4 * P), (max(4 * P, m * P), 8 * P)):
                    if lo >= hi:
                        continue
                    fo = lo - m * P  # offset into this chunk
                    nc.tensor.matmul(
                        ps[:, lo:hi],
                        x_sb[:, c : c + 1],
                        w_sb[:, fo : fo + (hi - lo)],
                        start=(c == 0),
                        stop=(c == NTILEC - 1 and lo >= 4 * P)
                        or (c == 4 * NSEG - 1 and lo < 4 * P),
                        skip_group_check=True,
                    )

            nc.scalar.copy(y_sb[:, 0 : 4 * P], ps[:, 0 : 4 * P])
            nc.scalar.copy(y_sb[:, 4 * P :], ps[:, 4 * P :])
            nc.sync.dma_start(out=y[:, :], in_=y_sb)

    nc.finalize()  # Bacc.compile(): wait splitting, reg alloc, etc.
    return nc


def _pack_core(W, k):
    """Build wt [128, 36864] for core k: column-major packed, transposed,
    tril-masked tiles. Chunk c holds tiles (g, c) for g = c//8 .. 7 with
    tile[j, i] = Wmask[128*(8g+k) + i, 128*c + j]."""
    # per-seg masked blocks, c-tiled and transposed: [C, j, i]
    seg_tiles = []
    for g in range(NSEG):
        i0 = P * (NSEG * g + k)
        width = P * NSEG * (g + 1)
        block = W[i0 : i0 + P, :width]
        cols = np.arange(width)[None, :]
        rows = (i0 + np.arange(P))[:, None]
        b = np.where(cols <= rows, block, np.float32(0.0)).astype(np.float32)
        c = width // P
        seg_tiles.append(b.reshape(P, c, P).transpose(1, 2, 0))  # [c, j, i]
    out = np.empty((P, WT_COLS), dtype=np.float32)
    for c in range(NTILEC):
        m = c // NSEG
        off = int(CHUNK_OFF[c])
        for g in range(m, NSEG):
            t = off + (g - m)
            out[:, P * t : P * (t + 1)] = seg_tiles[g][c]
    return out


def kernel(x, W, _trace=False, _trace_kwargs=None):
    global _compiled, last_results
    from concourse.bass_utils import run_bass_kernel_spmd

    x = np.asarray(x, dtype=np.float32)
    W = np.asarray(W, dtype=np.float32)

    if _compiled is None:
        _compiled = _build_program()

    xr = np.ascontiguousarray(x.reshape(NTILEC, P).T)
    in_maps = [{"wt": _pack_core(W, k), "xr": xr} for k in range(NCORES)]

    kw = {}
    if _trace:
        kw = dict(trace=True, **(_trace_kwargs or {}))
    res = run_bass_kernel_spmd(_compiled, in_maps, list(range(NCORES)), **kw)
    last_results = res

    y = np.empty(N, dtype=np.float32)
    for k in range(NCORES):
        yk = res.results[k]["y"].reshape(NSEG * P)
        for g in range(NSEG):
            i0 = P * (NSEG * g + k)
            y[i0 : i0 + P] = yk[P * g : P * (g + 1)]
    return y


# revision 24
# speedup vs baseline: 1.7465x; 1.0124x over previous
"""Lower-triangular matvec y = tril(W) @ x on 8 TRN2 NeuronCores.

Strategy (memory-bound problem; minimize + balance HBM traffic):
  - Split the 8192 rows into 64 row-tiles of 128. Row-tile t only needs
    columns [0, 128*(t+1)) of W (lower triangle).
  - Core k takes row-tiles {8g + k : g = 0..7} — one from each "width
    group" g, padded to the group max width 1024*(g+1). Every core
    therefore reads an identical 18.9MB (vs 32MB for naive row blocks),
    perfectly balanced, with SPMD-identical shapes.
  - The host applies the tril mask (zeros in the pad region) and stores
    tiles TRANSPOSED (contraction j on partitions), packed column-major:
    for each column-tile c, the tiles of all active segments g >= c//8
    sit contiguously. One DMA per c, perfectly strided.
  - PE usage: x columns are the STATIONARY operand (128-element weight
    loads) and W tiles stream as the moving operand, so each matmul does
    up to 512 columns of work per (tiny) weight load:
        ps[0, 128g + i] += sum_j x[128c+j] * W[128(8g+k)+i, 128c+j]
    96 matmuls total, accumulating y into a [1, 1024] PSUM row (two
    banks, matmuls never cross the 512 boundary).
  - 2 PSUM->SBUF copies, one [1, 1024] output DMA; host reassembles y.
"""

import numpy as np

P = 128
N = 8192
NCORES = 8
NSEG = 8  # row-tiles (segments) per core
NTILEC = 64  # column tiles over full width
# column-major packing: chunk c holds tiles for segs m..7, m = c//8
CHUNK_TILES = [NSEG - (c // NSEG) for c in range(NTILEC)]
CHUNK_OFF = np.concatenate([[0], np.cumsum(CHUNK_TILES)]).astype(int)  # in tiles
WT_COLS = int(CHUNK_OFF[-1]) * P  # 288 tiles * 128 = 36864

_compiled = None
last_results = None  # BassKernelResults of the most recent run (for test.py)


def _build_program():
    import concourse.bacc as bacc
    import concourse.mybir as mybir
    from concourse.tile import TileContext

    f32 = mybir.dt.float32
    f32r = mybir.dt.float32r
    # Bacc (not raw Bass): its compile() runs move_matmul_waits_to_ldweights
    # and generate_event_semaphores, which TRN2's 1-wait-per-instruction
    # ISA constraint requires for Tile-scheduled programs.
    nc = bacc.Bacc(None, target_bir_lowering=False)
    wt = nc.declare_dram_parameter("wt", [P, WT_COLS], f32r, isOutput=False)
    xr = nc.declare_dram_parameter("xr", [P, NTILEC], f32r, isOutput=False)
    y = nc.declare_dram_parameter("y", [1, NSEG * P], f32, isOutput=True)

    with TileContext(nc) as tc:
        with (
            tc.tile_pool(name="xpool", bufs=1) as xpool,
            tc.tile_pool(name="wpool", bufs=8) as wpool,
            tc.tile_pool(name="ypool", bufs=1) as ypool,
            tc.tile_pool(name="psum", bufs=1, space="PSUM") as psum_pool,
        ):
            x_sb = xpool.tile([P, NTILEC], f32r)
            nc.sync.dma_start(out=x_sb, in_=xr[:, :])
            y_sb = ypool.tile([1, NSEG * P], f32)

            ps = psum_pool.tile([1, NSEG * P], f32)  # two PSUM banks

            for c in range(NTILEC):
                m = c // NSEG  # first active segment
                w = CHUNK_TILES[c] * P  # chunk width (tiles for segs m..7)
                # all 64 chunks get their own buffer (no reuse waits);
                # same-m chunks share a tag so slots are sized exactly.
                w_sb = wpool.tile([P, w], f32r, tag=f"w{m}", bufs=NSEG, name=f"w_{c}")
                nc.sync.dma_start(
                    out=w_sb, in_=wt[:, P * CHUNK_OFF[c] : P * CHUNK_OFF[c + 1]]
                )
                # matmuls: out[0, 128g + i] over active segs; split at the
                # PSUM bank boundary (f32 bank = 512 elems).
                for lo, hi in ((m * P, 4 * P), (max(4 * P, m * P), 8 * P)):
                    if lo >= hi:
                        continue
                    fo = lo - m * P  # offset into this chunk
                    nc.tensor.matmul(
                        ps[:, lo:hi],
                        x_sb[:, c : c + 1],
                        w_sb[:, fo : fo + (hi - lo)],
                        start=(c == 0),
                        stop=(c == NTILEC - 1 and lo >= 4 * P)
                        or (c == 4 * NSEG - 1 and lo < 4 * P),
                        skip_group_check=True,
                    )

            nc.scalar.copy(y_sb[:, 0 : 4 * P], ps[:, 0 : 4 * P])
            nc.scalar.copy(y_sb[:, 4 * P :], ps[:, 4 * P :])
            nc.sync.dma_start(out=y[:, :], in_=y_sb)

    nc.finalize()  # Bacc.compile(): wait splitting, reg alloc, etc.
    return nc


def _pack_core(W, k):
    """Build wt [128, 36864] for core k: column-major packed, transposed,
    tril-masked tiles. Chunk c holds tiles (g, c) for g = c//8 .. 7 with
    tile[j, i] = Wmask[128*(8g+k) + i, 128*c + j]."""
    # per-seg masked blocks, c-tiled and transposed: [C, j, i]
    seg_tiles = []
    for g in range(NSEG):
        i0 = P * (NSEG * g + k)
        width = P * NSEG * (g + 1)
        block = W[i0 : i0 + P, :width]
        cols = np.arange(width)[None, :]
        rows = (i0 + np.arange(P))[:, None]
        b = np.where(cols <= rows, block, np.float32(0.0)).astype(np.float32)
        c = width // P
        seg_tiles.append(b.reshape(P, c, P).transpose(1, 2, 0))  # [c, j, i]
    out = np.empty((P, WT_COLS), dtype=np.float32)
    for c in range(NTILEC):
        m = c // NSEG
        off = int(CHUNK_OFF[c])
        for g in range(m, NSEG):
            t = off + (g - m)
            out[:, P * t : P * (t + 1)] = seg_tiles[g][c]
    return out


def kernel(x, W, _trace=False, _trace_kwargs=None):
    global _compiled, last_results
    from concourse.bass_utils import run_bass_kernel_spmd

    x = np.asarray(x, dtype=np.float32)
    W = np.asarray(W, dtype=np.float32)

    if _compiled is None:
        _compiled = _build_program()

    xr = np.ascontiguousarray(x.reshape(NTILEC, P).T)
    in_maps = [{"wt": _pack_core(W, k), "xr": xr} for k in range(NCORES)]

    kw = {}
    if _trace:
        kw = dict(trace=True, **(_trace_kwargs or {}))
    res = run_bass_kernel_spmd(_compiled, in_maps, list(range(NCORES)), **kw)
    last_results = res

    y = np.empty(N, dtype=np.float32)
    for k in range(NCORES):
        yk = res.results[k]["y"].reshape(NSEG * P)
        for g in range(NSEG):
            i0 = P * (NSEG * g + k)
            y[i0 : i0 + P] = yk[P * g : P * (g + 1)]
    return y
